# revision 7
# baseline (speedup 1.0000x reference)
"""Axial attention kernel for nn_AxialAttention_71734543778490 on 8 TRN2 cores.

Strategy: data-parallel over batch N=32 (4 images per NeuronCore), with the
whole forward implemented as a single hand-written Bass/Tile kernel per core
(no XLA graph, no 180MB [N,H,H,W,G] intermediates in HBM). Host<->device
traffic is fp16 (25.7MB each way instead of 51.4MB fp32).

Per-core kernel (NB=4 images), per image b and group g (H=W=56, G=8):
  q|k = x @ Wqk + b     (BN gammas folded into weights; k also carries the
                         qk-BN gamma, compensated inside the kr rel table)
  v   = x @ Wv + b      (sv-BN gamma folded)
  s[j,i,w] = qk + qr + kr   computed as:
    qk: 56 tiny per-w matmuls (contract c=8) into PSUM
    qr/kr: one [8x111] stationary matmul each -> G[m,(i,w)] in PSUM, stored
      to a skewed DRAM scratch (addr=(m+i)*3136+56i+w) so that the diagonal
      reindex qr[j,i,w]=G1[55+j-i,i,w] becomes an all-positive-stride 3D DMA
      gather (224B contiguous runs both sides).
  sim = softmax_w(s)  (f32 max/exp path, f16 storage)
  sv:  per-w matmuls against a j-major V^T gathered via a DRAM bounce
  sve: per-i matmuls against partition-windowed vrel tables
  out = PE-transpose back to [hw, 128] channels-last, fp16 store.
"""

import os
from concurrent.futures import ThreadPoolExecutor
from contextlib import ExitStack

import numpy as np

BN_EPS = 1e-3
H = 56
HW = H * H          # 3136
C = 128
G = 8
GC = 16
CQ = 8
A = 2 * H - 1       # 111
SKEW = A + H - 1    # 166
NB = 4              # images per core
NCORES = 8
N = 32

_POOL = ThreadPoolExecutor(max_workers=8)


def _prep_params(inputs):
    import numpy as np
    f = {k: np.asarray(v, np.float64) for k, v in inputs.items() if k != "x"}
    s = 1.0 / np.sqrt(1.0 + BN_EPS)
    gq = f["g_q"] * s
    gk = f["g_k"] * s
    gv = f["g_v"] * s
    gqk = f["g_qk"] * s
    gqr = f["g_qr"] * s
    gkr = f["g_kr"] * s
    gsv = f["g_sv"] * s
    gsve = f["g_sve"] * s

    wq = f["w_q"] * gq[None, :]
    kg = np.repeat(gqk, CQ)
    wk = f["w_k"] * (gk * kg)[None, :]
    bk = f["b_k"] * kg
    wv = f["w_v"] * (gv * gsv)[None, :]
    bv = f["b_v"] * gsv

    wqk = np.concatenate([wq, wk], axis=1)
    bqk = np.concatenate([f["b_q"], bk])[:, None]

    q_rel = f["q_rel"][:, 0, :]
    k_rel = f["k_rel"][:, 0, :]
    v_rel = f["v_rel"][:, 0, :]
    qrel = np.empty((CQ, G * A))
    krel = np.empty((CQ, G * A))
    rev = q_rel[::-1, :]
    revk = k_rel[::-1, :]
    for g in range(G):
        qrel[:, g * A:(g + 1) * A] = (rev * gqr[g]).T
        # k rows carry the folded qk-BN gamma; compensate it in the kr table
        krel[:, g * A:(g + 1) * A] = (revk * (gkr[g] / gqk[g])).T

    vrel = np.tile(v_rel, (1, G)) * gsve[None, :]
    outb = (f["b_sv"] + f["b_sve"]).reshape(G, GC).T

    return dict(
        wqk=wqk.astype(np.float16), wv=wv.astype(np.float16),
        bqk=bqk.astype(np.float32), bv=bv[:, None].astype(np.float32),
        qrel=qrel.astype(np.float16), krel=krel.astype(np.float16),
        vrel=vrel.astype(np.float16), outb=outb.astype(np.float32),
    )


_PARAM_NAMES = ["wqk", "wv", "bqk", "bv", "qrel", "krel", "vrel", "outb"]


def _axial_body(ctx, tc, out_ap, x_ap, p_aps, nb):
    import concourse.bass as bass
    from concourse import mybir, masks

    F16 = mybir.dt.float16
    F32 = mybir.dt.float32
    AF = mybir.ActivationFunctionType
    ALU = mybir.AluOpType
    AX = mybir.AxisListType
    nc = tc.nc

    cpool = ctx.enter_context(tc.tile_pool(name="consts", bufs=1))
    ident = cpool.tile([C, C], F16, tag="ident")
    masks.make_identity(nc, ident[:])
    wqk = cpool.tile([C, C], F16, tag="wqk")
    nc.sync.dma_start(wqk[:], p_aps["wqk"])
    wv = cpool.tile([C, C], F16, tag="wv")
    nc.sync.dma_start(wv[:], p_aps["wv"])
    bqk = cpool.tile([C, 1], F32, tag="bqk")
    nc.sync.dma_start(bqk[:], p_aps["bqk"])
    bv = cpool.tile([C, 1], F32, tag="bv")
    nc.sync.dma_start(bv[:], p_aps["bv"])
    qrel = cpool.tile([CQ, G * A], F16, tag="qrel")
    nc.sync.dma_start(qrel[:], p_aps["qrel"])
    krel = cpool.tile([CQ, G * A], F16, tag="krel")
    nc.sync.dma_start(krel[:], p_aps["krel"])
    outb = cpool.tile([GC, G], F32, tag="outb")
    nc.sync.dma_start(outb[:], p_aps["outb"])
    vrelw = cpool.tile([H, H * C], F16, tag="vrelw")
    for i in range(H):
        nc.sync.dma_start(vrelw[:, i * C:(i + 1) * C],
                          p_aps["vrel"][55 - i:111 - i, :])

    ps = ctx.enter_context(tc.tile_pool(name="ps", bufs=8, space="PSUM"))
    xst = ctx.enter_context(tc.tile_pool(name="xst", bufs=3))
    xtp = ctx.enter_context(tc.tile_pool(name="xtp", bufs=1))
    qkp = ctx.enter_context(tc.tile_pool(name="qkp", bufs=2))
    qgp = ctx.enter_context(tc.tile_pool(name="qgp", bufs=2))
    vp = ctx.enter_context(tc.tile_pool(name="vp", bufs=2))
    vtp = ctx.enter_context(tc.tile_pool(name="vtp", bufs=1))
    gst = ctx.enter_context(tc.tile_pool(name="gst", bufs=4))
    rp = ctx.enter_context(tc.tile_pool(name="rp", bufs=2))
    sp = ctx.enter_context(tc.tile_pool(name="sp", bufs=2))
    ep = ctx.enter_context(tc.tile_pool(name="ep", bufs=2))
    simp = ctx.enter_context(tc.tile_pool(name="simp", bufs=2))
    mzp = ctx.enter_context(tc.tile_pool(name="mzp", bufs=4))
    outp = ctx.enter_context(tc.tile_pool(name="outp", bufs=2))
    ogp = ctx.enter_context(tc.tile_pool(name="ogp", bufs=2))
    otp = ctx.enter_context(tc.tile_pool(name="otp", bufs=3))
    dram = ctx.enter_context(tc.tile_pool(name="dram", bufs=2, space="DRAM"))
    dramv = ctx.enter_context(tc.tile_pool(name="dramv", bufs=2, space="DRAM"))

    NCH = 28
    CHW = 112
    NWB = 7
    CHF = 448

    for b in range(nb):
        xt = xtp.tile([C, HW], F16, tag="xt")
        for ch in range(NCH):
            xs = xst.tile([CHW, C], F16, tag="xs")
            nc.sync.dma_start(xs[:], x_ap[b, ch * CHW:(ch + 1) * CHW, :])
            pt = ps.tile([C, CHW], F16, tag="ps")
            nc.tensor.transpose(pt[:], xs[:], ident[:CHW, :CHW])
            nc.scalar.copy(xt[:, ch * CHW:(ch + 1) * CHW], pt[:])

        qk_sb = qkp.tile([C, HW], F16, tag="qk_sb")
        v_sb = vp.tile([C, HW], F16, tag="v_sb")
        for ck in range(NWB):
            sl = slice(ck * CHF, (ck + 1) * CHF)
            pq = ps.tile([C, CHF], F32, tag="ps")
            nc.tensor.matmul(pq[:], wqk[:], xt[:, sl], start=True, stop=True)
            nc.scalar.add(qk_sb[:, sl], pq[:], add=bqk[:])
            pv = ps.tile([C, CHF], F32, tag="ps")
            nc.tensor.matmul(pv[:], wv[:], xt[:, sl], start=True, stop=True)
            nc.scalar.add(v_sb[:, sl], pv[:], add=bv[:])

        vd = dramv.tile([C, HW], F16, tag="vd")
        nc.sync.dma_start(vd[:], v_sb[:])
        vt = vtp.tile([H, G * GC * H], F16, tag="vt")
        vd_t = vd[:].tensor
        vd_off = vd[:].offset
        src = bass.AP(vd_t, vd_off,
                      [[H, H], [GC * HW, G], [HW, GC], [1, H]])
        vt_view = vt[:].rearrange("j (g c w) -> j g c w", g=G, c=GC, w=H)
        nc.sync.dma_start(vt_view, src)

        out_all = outp.tile([C, HW], F16, tag="out_all")

        for g in range(G):
            qrow = g * CQ
            vrow = g * GC

            qg = qgp.tile([CQ, HW], F16, tag="qg")
            nc.sync.dma_start(qg[:], qk_sb[qrow:qrow + CQ, :])
            kg = qgp.tile([CQ, HW], F16, tag="qg")
            nc.sync.dma_start(kg[:], qk_sb[64 + qrow:64 + qrow + CQ, :])

            g1d = dram.tile([SKEW, HW], F16, tag="g1d")
            g1_t = g1d[:].tensor
            g1_off = g1d[:].offset
            for ck in range(NWB):
                i0 = ck * 8
                pg = ps.tile([A, CHF], F32, tag="ps")
                nc.tensor.matmul(
                    pg[:], qrel[:, g * A:(g + 1) * A],
                    qg[:, i0 * H:(i0 + 8) * H], start=True, stop=True)
                gs = gst.tile([A, CHF], F16, tag="gs")
                nc.scalar.copy(gs[:], pg[:])
                dst = bass.AP(g1_t, g1_off + i0 * (HW + H),
                              [[HW, A], [HW + H, 8], [1, H]])
                nc.sync.dma_start(dst, gs[:].rearrange(
                    "m (i w) -> m i w", i=8, w=H))

            g2d = dram.tile([SKEW, HW], F16, tag="g2d")
            g2_t = g2d[:].tensor
            g2_off = g2d[:].offset
            for ck in range(NWB):
                j0 = ck * 8
                pg = ps.tile([A, CHF], F32, tag="ps")
                nc.tensor.matmul(
                    pg[:], krel[:, g * A:(g + 1) * A],
                    kg[:, j0 * H:(j0 + 8) * H], start=True, stop=True)
                gs = gst.tile([A, CHF], F16, tag="gs")
                nc.scalar.copy(gs[:], pg[:])
                dst = bass.AP(g2_t, g2_off + j0 * (HW + H),
                              [[HW, A], [HW + H, 8], [1, H]])
                nc.sync.dma_start(dst, gs[:].rearrange(
                    "m (j w) -> m j w", j=8, w=H))

            r1 = rp.tile([H, HW], F16, tag="r1")
            src1 = bass.AP(g1_t, g1_off + 55 * HW,
                           [[HW, H], [H, H], [1, H]])
            nc.sync.dma_start(r1[:].rearrange("j (i w) -> j i w", i=H, w=H),
                              src1)
            r2 = rp.tile([H, HW], F16, tag="r2")
            src2 = bass.AP(g2_t, g2_off + 55 * HW,
                           [[H, H], [HW, H], [1, H]])
            nc.sync.dma_start(r2[:].rearrange("j (i w) -> j i w", i=H, w=H),
                              src2)
            r12 = rp.tile([H, HW], F16, tag="r12")
            nc.vector.tensor_tensor(out=r12[:], in0=r1[:], in1=r2[:],
                                    op=ALU.add)

            S = sp.tile([H, HW], F32, tag="S")
            q3 = qg[:].rearrange("p (i w) -> p i w", i=H, w=H)
            k3 = kg[:].rearrange("p (j w) -> p j w", j=H, w=H)
            for wb in range(NWB):
                pk = ps.tile([H, CHF], F32, tag="ps")
                for wl in range(8):
                    w = wb * 8 + wl
                    nc.tensor.matmul(
                        pk[:, wl * H:(wl + 1) * H],
                        k3[:, :, w], q3[:, :, w], start=True, stop=True)
                s_wi = S[:].rearrange("j (i w) -> j w i", i=H, w=H)
                r_wi = r12[:].rearrange("j (i w) -> j w i", i=H, w=H)
                nc.vector.tensor_tensor(
                    out=s_wi[:, wb * 8:(wb + 1) * 8, :],
                    in0=pk[:].rearrange("j (w i) -> j w i", w=8, i=H),
                    in1=r_wi[:, wb * 8:(wb + 1) * 8, :],
                    op=ALU.add)

            s3 = S[:].rearrange("j (i w) -> j i w", i=H, w=H)
            mneg = mzp.tile([H, H], F32, tag="mneg")
            nc.vector.tensor_reduce(mneg[:], s3, axis=AX.X, op=ALU.max,
                                    negate=True)
            mb = mneg[:].broadcast_to((H, H, H))
            nc.vector.tensor_tensor(out=s3, in0=s3, in1=mb, op=ALU.add)
            et = ep.tile([H, HW], F16, tag="E")
            nc.scalar.activation(et[:], S[:], AF.Exp)
            z = mzp.tile([H, H], F32, tag="z")
            e3 = et[:].rearrange("j (i w) -> j i w", i=H, w=H)
            nc.vector.tensor_reduce(z[:], e3, axis=AX.X, op=ALU.add)
            rz = mzp.tile([H, H], F32, tag="rz")
            nc.vector.reciprocal(rz[:], z[:])
            sim = simp.tile([H, HW], F16, tag="sim")
            sim3 = sim[:].rearrange("j (i w) -> j i w", i=H, w=H)
            rzb = rz[:].broadcast_to((H, H, H))
            nc.vector.tensor_tensor(out=sim3, in0=e3, in1=rzb, op=ALU.mult)

            vt4 = vt[:].rearrange("j (g c w) -> j g c w", g=G, c=GC, w=H)
            sim_iw = sim[:].rearrange("j (i w) -> j i w", i=H, w=H)
            outg = ogp.tile([GC, HW], F16, tag="outg")
            og3 = outg[:].rearrange("p (i w) -> p w i", i=H, w=H)
            for wb in range(NWB):
                psv = ps.tile([GC, CHF], F32, tag="ps")
                for wl in range(8):
                    w = wb * 8 + wl
                    nc.tensor.matmul(
                        psv[:, wl * H:(wl + 1) * H],
                        vt4[:, g, :, w], sim_iw[:, :, w],
                        start=True, stop=True)
                nc.scalar.add(
                    og3[:, wb * 8:(wb + 1) * 8, :],
                    psv[:].rearrange("c (w i) -> c w i", w=8, i=H),
                    add=outb[:, g:g + 1])

            for ib in range(NWB):
                pse = ps.tile([GC, CHF], F32, tag="ps")
                for il in range(8):
                    i = ib * 8 + il
                    nc.tensor.matmul(
                        pse[:, il * H:(il + 1) * H],
                        vrelw[:, i * C + vrow:i * C + vrow + GC],
                        sim[:, i * H:(i + 1) * H], start=True, stop=True)
                osl = outg[:, ib * 8 * H:(ib + 1) * 8 * H]
                nc.vector.tensor_tensor(out=osl, in0=pse[:], in1=osl,
                                        op=ALU.add)
            nc.sync.dma_start(out_all[vrow:vrow + GC, :], outg[:])

        for ch in range(NCH):
            pt = ps.tile([CHW, C], F16, tag="ps")
            nc.tensor.transpose(pt[:], out_all[:, ch * CHW:(ch + 1) * CHW],
                                ident[:])
            ot = otp.tile([CHW, C], F16, tag="ot")
            nc.scalar.copy(ot[:], pt[:])
            nc.sync.dma_start(out_ap[b, ch * CHW:(ch + 1) * CHW, :], ot[:])


_STATE = None


def _build():
    global _STATE
    if _STATE is not None:
        return _STATE

    import jax
    import concourse.bass as bass
    import concourse.tile as tile
    from concourse import bacc, mybir
    from concourse import bass2jax
    from concourse.bass2jax import _bass_exec_p, install_neuronx_cc_hook
    from jax.experimental.shard_map import shard_map
    from jax.sharding import Mesh, PartitionSpec

    F16 = mybir.dt.float16
    F32 = mybir.dt.float32

    nc = bacc.Bacc("TRN2", target_bir_lowering=False, debug=False,
                   num_devices=NCORES)
    x_d = nc.dram_tensor("x", [NB, HW, C], F16, kind="ExternalInput")
    pshapes = {
        "wqk": ([C, C], F16), "wv": ([C, C], F16),
        "bqk": ([C, 1], F32), "bv": ([C, 1], F32),
        "qrel": ([CQ, G * A], F16), "krel": ([CQ, G * A], F16),
        "vrel": ([A, C], F16), "outb": ([GC, G], F32),
    }
    p_d = {k: nc.dram_tensor(k, sh, dt, kind="ExternalInput")
           for k, (sh, dt) in pshapes.items()}
    out_d = nc.dram_tensor("out", [NB, HW, C], F16, kind="ExternalOutput")

    with tile.TileContext(nc) as tc:
        with ExitStack() as ctx:
            _axial_body(ctx, tc, out_d.ap(), x_d.ap(),
                        {k: v.ap() for k, v in p_d.items()}, NB)
    nc.compile()

    install_neuronx_cc_hook()

    partition_name = (nc.partition_id_tensor.name
                      if nc.partition_id_tensor else None)
    in_names = []
    out_names = []
    out_avals = []
    zero_outs = []
    for alloc in nc.m.functions[0].allocations:
        if not isinstance(alloc, mybir.MemoryLocationSet):
            continue
        name = alloc.memorylocations[0].name
        if alloc.kind == "ExternalInput":
            if name != partition_name:
                in_names.append(name)
        elif alloc.kind == "ExternalOutput":
            shape = tuple(alloc.tensor_shape)
            dtype = mybir.dt.np(alloc.dtype)
            out_names.append(name)
            out_avals.append(jax.core.ShapedArray(shape, dtype))
            zero_outs.append(np.zeros((NCORES * shape[0], *shape[1:]), dtype))
    n_params = len(in_names)
    param_order = list(in_names)
    all_names = in_names + out_names
    if partition_name is not None:
        all_names = all_names + [partition_name]
    donate = tuple(range(n_params, n_params + len(out_names)))

    def _body(*args):
        operands = list(args)
        if partition_name is not None:
            operands.append(bass2jax.partition_id_tensor())
        outs = _bass_exec_p.bind(
            *operands,
            out_avals=tuple(out_avals),
            in_names=tuple(all_names),
            out_names=tuple(out_names),
            lowering_input_output_aliases=(),
            sim_require_finite=True,
            sim_require_nnan=True,
            nc=nc,
        )
        return tuple(outs)

    devices = jax.devices()[:NCORES]
    mesh = Mesh(np.asarray(devices), ("core",))
    nio = n_params + len(out_names)
    sharded = jax.jit(
        shard_map(_body, mesh=mesh,
                  in_specs=(PartitionSpec("core"),) * nio,
                  out_specs=(PartitionSpec("core"),) * len(out_names),
                  check_rep=False),
        donate_argnums=donate, keep_unused=True)

    # output scratch buffers created device-side (no 25MB host->device zeros)
    import jax.numpy as jnp
    from jax.sharding import NamedSharding
    zshardings = [NamedSharding(mesh, PartitionSpec("core"))] * len(zero_outs)
    zshapes = [(z.shape, z.dtype) for z in zero_outs]

    def _mk_zeros():
        return tuple(jnp.zeros(sh, dt) for sh, dt in zshapes)

    mk_zeros = jax.jit(_mk_zeros, out_shardings=tuple(zshardings))

    _STATE = (sharded, param_order, out_names, mk_zeros)
    return _STATE


def kernel(**inputs) -> np.ndarray:
    import time
    dbg = os.environ.get("AXIAL_TIMING")
    t0 = time.perf_counter()
    sharded, param_order, out_names, mk_zeros = _build()
    t1 = time.perf_counter()

    params = _prep_params(inputs)

    x = np.asarray(inputs["x"])
    x = np.ascontiguousarray(x).reshape(N, HW, C)
    # threaded fp32 -> fp16 cast of the big input
    x16 = np.empty((N, HW, C), np.float16)

    def _cast(i):
        x16[i * 4:(i + 1) * 4] = x[i * 4:(i + 1) * 4]
    list(_POOL.map(_cast, range(8)))

    concat_in = []
    for name in param_order:
        if name == "x":
            concat_in.append(x16)  # [32, HW, C] -> 8 shards of [4, HW, C]
        else:
            p = params[name]
            concat_in.append(np.concatenate([p] * NCORES, axis=0))
    t2 = time.perf_counter()

    out_arrs = sharded(*concat_in, *mk_zeros())
    for o in out_arrs:
        o.block_until_ready()
    t3 = time.perf_counter()
    out = np.asarray(out_arrs[0])              # [32, HW, C] f16
    t4 = time.perf_counter()
    res = out.reshape(N, H, H, C).astype(np.float32)
    if dbg:
        print(f"[axial] build {t1-t0:.3f}s prep {t2-t1:.3f}s "
              f"h2d+exec {t3-t2:.3f}s d2h {t4-t3:.3f}s "
              f"cast {time.perf_counter()-t4:.3f}s", flush=True)
    return res
